# revision 15
# baseline (speedup 1.0000x reference)
"""EGNN encoder kernel for Trainium2 (Bass/Tile), 8-core SPMD.

Strategy:
  - Edge-parallel: core c owns destination rows [c*ROWN, (c+1)*ROWN).
  - Edges sorted by row globally; per core, edges grouped into 128-row
    blocks; within a block edges are sorted by col and split at col SPLIT
    so dma_gather's int16 indices work in two passes.
  - Per layer, node tables tabA=[h@W1a+b1 | coord] (own rows, local) and
    tabB=[h@W1b | coord] (all rows) live in DRAM with 64-float rows
    (256B, dma_gather elem constraint).  tabB content is exchanged via a
    compact [ROWN,36] AllGather (ping-pong input buffers to avoid
    straggler overwrite races) and repacked locally into the 256B-row
    gather table.
  - Edge MLP runs feat-major on PE via batched transposes (2 groups of 4
    subtiles share each matmul); segment-sum via one-hot matmuls
    accumulating into PSUM per 128-row block.
"""

import math
import os
from contextlib import ExitStack
from dataclasses import dataclass, field

PROBE_NO_COLL = bool(int(os.environ.get("EGNN_NO_COLL", "0")))
PROBE_NO_GATHER = bool(int(os.environ.get("EGNN_NO_GATHER", "0")))

import numpy as np

import concourse.bass as bass
import concourse.tile as tile
from concourse import mybir
from concourse.bass import AP
from concourse.masks import make_identity

F32 = mybir.dt.float32
BF16 = mybir.dt.bfloat16
I32 = mybir.dt.int32
I16 = mybir.dt.int16
ALU = mybir.AluOpType
ACTF = mybir.ActivationFunctionType

NC = 8        # cores
H = 32        # hidden
NL = 4        # layers
TW = 64       # table row width (floats) = 256B for dma_gather
CW = 36       # compact row width (h 32 + coord 4)


@dataclass
class Cfg:
    N: int
    E: int
    G: int
    L: float = 10.0
    EPS: float = 1e-8
    SPLIT: int = 32768  # int16 index limit for col gathers
    # derived
    ROWN: int = 0
    BBLK: int = 0
    RPAD: int = 0
    NPAD: int = 0
    S1: int = 0
    S2: int = 0
    S: int = 0
    CHUNKS: list = field(default_factory=list)
    wslots: dict = field(default_factory=dict)
    WC: int = 0

    def derive_static(self):
        self.ROWN = self.N // NC
        self.BBLK = (self.ROWN + 127) // 128
        self.RPAD = self.BBLK * 128
        self.NPAD = ((self.N + 127) // 128) * 128
        # dense node-phase chunks (512 then remainder)
        self.CHUNKS = []
        o = 0
        while o < self.RPAD:
            w = min(512, self.RPAD - o)
            self.CHUNKS.append((o, w))
            o += w


# ---------------------------------------------------------------- host pre

def preprocess(inp, cfg: Cfg):
    """Sort/shard edges, build per-core index arrays and weight pack."""
    cfg.derive_static()
    N, E, G = cfg.N, cfg.E, cfg.G
    ROWN, BBLK = cfg.ROWN, cfg.BBLK

    row = np.asarray(inp["edge_index"][0]).astype(np.int64)
    col = np.asarray(inp["edge_index"][1]).astype(np.int64)
    order = np.argsort(row, kind="stable")
    rs, cs = row[order], col[order]

    # per-core, per-block edge lists (block = 128 dest rows), col-sorted
    core_edges = []  # [core][block] -> (r_loc, c_gl, nlo) col-split
    S1 = S2 = 0
    for c in range(NC):
        lo = np.searchsorted(rs, c * ROWN)
        hi = np.searchsorted(rs, (c + 1) * ROWN)
        r_loc = rs[lo:hi] - c * ROWN
        c_gl = cs[lo:hi]
        blk = r_loc // 128
        blocks = []
        for b in range(BBLK):
            blo = np.searchsorted(blk, b)
            bhi = np.searchsorted(blk, b + 1)
            rb, cb = r_loc[blo:bhi], c_gl[blo:bhi]
            o2 = np.argsort(cb, kind="stable")
            rb, cb = rb[o2], cb[o2]
            nlo = int(np.searchsorted(cb, cfg.SPLIT))
            nhi = len(cb) - nlo
            S1 = max(S1, (nlo + 127) // 128)
            S2 = max(S2, (nhi + 127) // 128)
            blocks.append((rb, cb, nlo))
        core_edges.append(blocks)
    S1 = max(S1, 1)
    cfg.S1, cfg.S2, cfg.S = S1, S2, S1 + S2
    S = cfg.S

    def wrap16(a_i16):
        # [BBLK, S*128] entry i -> partition i%16 col i//16, replicated x8
        x = a_i16.reshape(BBLK, S * 128 // 16, 16)  # [b, col, p]
        x = x.transpose(2, 0, 1).reshape(16, BBLK * S * 8)
        return np.tile(x, (8, 1)).astype(np.int16)  # [128, BBLK*S*8]

    per_core = []
    batch = np.asarray(inp["batch"]).astype(np.int64)
    gcnt = np.bincount(batch, minlength=G).astype(np.float64)
    invg = (1.0 / np.maximum(gcnt, 1.0)).astype(np.float32)
    pos = np.asarray(inp["pos"]).astype(np.float32)
    x_in = np.asarray(inp["x"]).astype(np.float32).reshape(-1)

    for c in range(NC):
        idxr = np.zeros((BBLK, S * 128), np.int16)
        idxc = np.zeros((BBLK, S * 128), np.int16)
        rloc = np.full((BBLK, S * 128), -1.0, np.float32)
        cnt = np.zeros(cfg.RPAD, np.float64)
        for b in range(BBLK):
            rb, cb, nlo = core_edges[c][b]
            np.add.at(cnt, rb, 1.0)
            # lo edges at entries [0, nlo), hi edges at [S1*128, S1*128+nhi)
            idxr[b, :nlo] = rb[:nlo]
            idxc[b, :nlo] = cb[:nlo]
            rloc[b, :nlo] = (rb[:nlo] - 128 * b).astype(np.float32)
            nhi = len(rb) - nlo
            o = S1 * 128
            idxr[b, o:o + nhi] = rb[nlo:]
            idxc[b, o:o + nhi] = cb[nlo:] - cfg.SPLIT
            rloc[b, o:o + nhi] = (rb[nlo:] - 128 * b).astype(np.float32)
        d = {}
        if bool(int(os.environ.get("EGNN_ZERO_IDX", "0"))):
            idxr = np.zeros_like(idxr)
            idxc = np.zeros_like(idxc)
        wr, wc = wrap16(idxr), wrap16(idxc)
        # merged stream: per block [idxr S*8 | idxc S*8]
        m = np.zeros((128, BBLK * S * 16), np.int16)
        for b in range(BBLK):
            m[:, b * S * 16:b * S * 16 + S * 8] = wr[:, b * S * 8:(b + 1) * S * 8]
            m[:, b * S * 16 + S * 8:(b + 1) * S * 16] = wc[:, b * S * 8:(b + 1) * S * 8]
        d["idxrc"] = m
        # rloc layout: [128, BBLK*S], entry e=(sub*128+p) -> [p, b*S+sub]
        d["rloc"] = (
            rloc.reshape(BBLK, S, 128).transpose(2, 0, 1).reshape(128, BBLK * S)
        ).astype(np.float32).copy()
        inv = (1.0 / np.maximum(cnt, 1.0)).astype(np.float32)
        d["invc"] = inv.reshape(BBLK, 128).T.copy()  # [128, BBLK]
        # pos node-major [128, BBLK*4] (4th component stays 0)
        p4 = np.zeros((cfg.RPAD, 4), np.float32)
        p4[:ROWN, :3] = pos[c * ROWN:(c + 1) * ROWN]
        d["posnm"] = p4.reshape(BBLK, 128, 4).transpose(1, 0, 2).reshape(128, BBLK * 4).copy()
        # x2 [2, RPAD] rows: x, ones
        x2 = np.zeros((2, cfg.RPAD), np.float32)
        x2[0, :ROWN] = x_in[c * ROWN:(c + 1) * ROWN]
        x2[1, :] = 1.0
        d["x2"] = x2
        # weighted graph one-hot node-major [128, BBLK*G]
        goh = np.zeros((cfg.RPAD, G), np.float32)
        bloc = batch[c * ROWN:(c + 1) * ROWN]
        goh[np.arange(ROWN), bloc] = invg[bloc]
        d["goh"] = goh.reshape(BBLK, 128, G).transpose(1, 0, 2).reshape(128, BBLK * G).copy()
        per_core.append(d)

    # ---- weight pack (shared across cores) --------------------------------
    slots = {}
    colp = [0]
    wpack_parts = []

    def blkdiag(w, n=4):
        k, m = w.shape
        out = np.zeros((n * k, n * m), np.float32)
        for i in range(n):
            out[i * k:(i + 1) * k, i * m:(i + 1) * m] = w
        return out

    def add(name, arr):
        arr = np.asarray(arr, np.float32)
        assert arr.ndim == 2 and arr.shape[0] <= 128
        c0 = colp[0]
        slots[name] = (arr.shape[0], c0, arr.shape[1])
        colp[0] += arr.shape[1]
        wpack_parts.append((c0, arr))

    g = lambda k: np.asarray(inp[k], np.float32)
    add("EMB", np.vstack([g("emb_in_w"), g("emb_in_b")[None, :]]))
    for l in range(NL):
        w1 = g("edge_w1")[l]; b1 = g("edge_b1")[l]
        ab = np.zeros((33, 64), np.float32)
        ab[:32, :32] = w1[0:32]; ab[32, :32] = b1
        ab[:32, 32:] = w1[32:64]
        add(f"AB{l}", ab)
        add(f"w1c{l}", np.tile(w1[64:65, :], (128, 1)))
        add(f"W2{l}", blkdiag(g("edge_w2")[l]))
        add(f"b2{l}", np.tile(g("edge_b2")[l], 4)[:, None])
        add(f"C1{l}", blkdiag(g("coord_w1")[l]))
        add(f"c1{l}", np.tile(g("coord_b1")[l], 4)[:, None])
        add(f"C2{l}", blkdiag(g("coord_w2")[l]))  # [128, 4]
        add(f"c2{l}", np.full((128, 1), float(g("coord_b2")[l][0]), np.float32))
        add(f"N1h{l}", np.vstack([g("node_w1")[l][0:32], g("node_b1")[l][None, :]]))
        add(f"N1m{l}", g("node_w1")[l][32:64])
        add(f"N2{l}", np.vstack([g("node_w2")[l], g("node_b2")[l][None, :]]))
    add("EOUT", np.vstack([g("emb_out_w"), g("emb_out_b")[None, :]]))
    add("ZMU", np.vstack([g("zmu_w"), g("zmu_b")[None, :]]))
    add("ZSIG", np.vstack([g("zsig_w"), g("zsig_b")[None, :]]))
    cfg.WC = colp[0]
    cfg.wslots = slots
    wpack = np.zeros((128, cfg.WC), np.float32)
    for c0, arr in wpack_parts:
        wpack[: arr.shape[0], c0:c0 + arr.shape[1]] = arr

    for d in per_core:
        d["wpack"] = wpack
    return per_core


def input_specs(cfg: Cfg):
    """name -> (shape, np dtype) for per-core device inputs."""
    S, BBLK, RPAD, G = cfg.S, cfg.BBLK, cfg.RPAD, cfg.G
    return {
        "idxrc": ((128, BBLK * S * 16), np.int16),
        "rloc": ((128, BBLK * S), np.float32),
        "invc": ((128, BBLK), np.float32),
        "posnm": ((128, BBLK * 4), np.float32),
        "x2": ((2, RPAD), np.float32),
        "goh": ((128, BBLK * G), np.float32),
        "wpack": ((128, cfg.WC), np.float32),
    }


# ---------------------------------------------------------------- builder

def build(tc, outs, ins, cfg: Cfg):
    nc = tc.nc
    S, S1, S2 = cfg.S, cfg.S1, cfg.S2
    BBLK, RPAD, NPAD, G = cfg.BBLK, cfg.RPAD, cfg.NPAD, cfg.G
    N, ROWN = cfg.N, cfg.ROWN
    CHUNKS = cfg.CHUNKS
    S4 = (S + 3) // 4
    inv_l = 1.0 / cfg.L

    mu_o, sig_o = outs["mu"], outs["sig"]

    tabA = nc.dram_tensor("tabA", [RPAD, TW], F32, kind="Internal")
    tabBoc = [nc.dram_tensor(f"tabBoc{p}", [RPAD, CW], F32, kind="Internal")
              for p in range(2)]
    tabBc = nc.dram_tensor("tabBc", [N, CW], F32, kind="Internal",
                           addr_space="Shared")
    tabB = nc.dram_tensor("tabB", [NPAD, TW], F32, kind="Internal")
    gsin = nc.dram_tensor("gsin", [32, G], F32, kind="Internal")
    gsout = nc.dram_tensor("gsout", [32, G], F32, kind="Internal",
                           addr_space="Shared")

    ctx = ExitStack()
    with ctx:
        wp = ctx.enter_context(tc.tile_pool(name="wp", bufs=1))
        ep = ctx.enter_context(tc.tile_pool(name="ep", bufs=2))
        tp = ctx.enter_context(tc.tile_pool(name="tp", bufs=2))
        pse = ctx.enter_context(tc.tile_pool(name="pse", bufs=2, space="PSUM"))
        psphi = ctx.enter_context(tc.tile_pool(name="psphi", bufs=1, space="PSUM"))
        psagg = ctx.enter_context(tc.tile_pool(name="psagg", bufs=1, space="PSUM"))
        psn = ctx.enter_context(tc.tile_pool(name="psn", bufs=2, space="PSUM"))

        # ---------------- persistent state ----------------
        wt = wp.tile([128, cfg.WC], F32, tag="wt")
        nc.sync.dma_start(wt[:], ins["wpack"][:])

        def W(name):
            p, c0, w = cfg.wslots[name]
            return wt[0:p, c0:c0 + w]

        ident = wp.tile([128, 128], F32, tag="ident")
        make_identity(nc, ident[:])
        identb = wp.tile([128, 128], BF16, tag="identb")
        nc.vector.tensor_copy(identb[:], ident[:])
        w2b, c1b, c2b = [], [], []
        for l in range(NL):
            w2b.append(wp.tile([128, 128], BF16, tag=f"w2b{l}", name=f"w2b{l}"))
            c1b.append(wp.tile([128, 128], BF16, tag=f"c1b{l}", name=f"c1b{l}"))
            c2b.append(wp.tile([128, 4], BF16, tag=f"c2b{l}", name=f"c2b{l}"))
        iota = wp.tile([128, 128], I32, tag="iotai")
        nc.gpsimd.iota(iota[:], pattern=[[1, 128]], base=0, channel_multiplier=0)
        iotab = wp.tile([128, 128], BF16, tag="iotab")
        nc.vector.tensor_copy(iotab[:], iota[:])

        for l in range(NL):
            nc.vector.tensor_copy(w2b[l][:], W(f"W2{l}"))
            nc.vector.tensor_copy(c1b[l][:], W(f"C1{l}"))
            nc.vector.tensor_copy(c2b[l][:], W(f"C2{l}"))

        invc = wp.tile([128, BBLK], F32, tag="invc")
        nc.sync.dma_start(invc[:], ins["invc"][:])

        hT = wp.tile([33, RPAD], F32, tag="hT")
        nc.vector.memset(hT[32:33, :], 1.0)
        coord = wp.tile([128, BBLK, 4], F32, tag="coord")
        nc.sync.dma_start(coord[:], ins["posnm"][:].rearrange("p (b c) -> p b c", c=4))
        aggT = wp.tile([36, RPAD], F32, tag="aggT")
        tsnm = wp.tile([128, BBLK, 4], F32, tag="tsnm")

        # zero the pad regions of the DRAM tables once: columns CW:TW of
        # every row, plus tabB rows N:NPAD entirely.
        zt = wp.tile([128, TW], F32, tag="zt")
        nc.vector.memset(zt[:], 0.0)
        eps16 = wp.tile([128, 1], F32, tag="eps16")
        nc.vector.memset(eps16[:], 1e-16)

        def bc_mid(ap_, count):
            # [P, W] -> [P, count, W] with stride-0 middle dim
            return AP(ap_.tensor, ap_.offset, [ap_.ap[0], [0, count], ap_.ap[1]])

        nc.sync.dma_start(
            tabA[:].rearrange("(b p) c -> p b c", p=128)[:, :, CW:TW],
            bc_mid(zt[:, 0:TW - CW], RPAD // 128))
        nc.sync.dma_start(
            tabB[:].rearrange("(b p) c -> p b c", p=128)[:, :, CW:TW],
            bc_mid(zt[:, 0:TW - CW], NPAD // 128))
        if NPAD > N:
            nc.sync.dma_start(tabB[N:NPAD, 0:CW], zt[0:NPAD - N, 0:CW])

        # h = emb_in(x): hT[0:32] = EMB.T @ x2
        for o, w in CHUNKS:
            sl = slice(o, o + w)
            x2c = tp.tile([2, 512], F32, tag="x2c")
            nc.sync.dma_start(x2c[:, 0:w], ins["x2"][:, sl])
            ps = psn.tile([32, 512], F32, tag="psn")
            nc.tensor.matmul(ps[:, 0:w], lhsT=W("EMB"), rhs=x2c[:, 0:w])
            nc.scalar.copy(hT[0:32, sl], ps[:, 0:w])

        # ================= layers =================
        for l in range(NL):
            pp_ = l % 2
            # ---- write tabA (local) and tabBoc (collective input) ----
            for b in range(BBLK):
                bsl = slice(b * 128, (b + 1) * 128)
                ps = psn.tile([128, 64], F32, tag="psn")
                nc.tensor.matmul(ps[:], lhsT=hT[:, bsl], rhs=W(f"AB{l}"))
                tba = tp.tile([128, CW], F32, tag="tba")
                nc.scalar.copy(tba[:, 0:32], ps[:, 0:32])
                nc.vector.tensor_copy(tba[:, 32:36], coord[:, b, 0:4])
                nc.sync.dma_start(
                    tabA[:].rearrange("(b p) c -> p b c", p=128)[:, b, 0:CW],
                    tba[:])
                tbb = tp.tile([128, CW], F32, tag="tbb")
                nc.scalar.copy(tbb[:, 0:32], ps[:, 32:64])
                nc.vector.tensor_copy(tbb[:, 32:36], coord[:, b, 0:4])
                nc.sync.dma_start(
                    tabBoc[pp_][:].rearrange("(b p) c -> p b c", p=128)[:, b, :],
                    tbb[:])
            if not PROBE_NO_COLL:
                nc.gpsimd.collective_compute(
                    "AllGather", ALU.bypass,
                    replica_groups=[list(range(NC))],
                    ins=[tabBoc[pp_][0:ROWN, :]],
                    outs=[tabBc[0:N, :]],
                )
            # repack compact rows into the 256B-row gather table
            nc.sync.dma_start(tabB[0:N, 0:CW], tabBc[0:N, :])

            # ---- edge phase ----
            for b in range(BBLK):
                # stream indices for this block
                ixrc = ep.tile([128, S * 16], I16, tag="ixrc")
                nc.sync.dma_start(ixrc[:], ins["idxrc"][:, b * S * 16:(b + 1) * S * 16])
                rlocb = ep.tile([128, S], F32, tag="rlocb")
                nc.sync.dma_start(rlocb[:], ins["rloc"][:, b * S:(b + 1) * S])
                rl16 = ep.tile([128, S], BF16, tag="rl16")
                nc.vector.tensor_copy(rl16[:], rlocb[:])

                gr = ep.tile([128, S, TW], F32, tag="gr")
                if not PROBE_NO_GATHER:
                  nc.gpsimd.dma_gather(
                    out_ap=gr[:], in_ap=tabA[:], idxs_ap=ixrc[:, 0:S * 8],
                    num_idxs=S * 128, num_idxs_reg=S * 128, elem_size=TW,
                    single_packet=False)
                gc = ep.tile([128, S, TW], F32, tag="gc")
                if not PROBE_NO_GATHER:
                  nc.gpsimd.dma_gather(
                    out_ap=gc[:, 0:S1, :], in_ap=tabB[0:min(cfg.SPLIT, NPAD), :],
                    idxs_ap=ixrc[:, S * 8:S * 8 + S1 * 8],
                    num_idxs=S1 * 128, num_idxs_reg=S1 * 128, elem_size=TW,
                    single_packet=False)
                  if S2 > 0:
                    nc.gpsimd.dma_gather(
                        out_ap=gc[:, S1:S, :], in_ap=tabB[cfg.SPLIT:NPAD, :],
                        idxs_ap=ixrc[:, S * 8 + S1 * 8:S * 16],
                        num_idxs=S2 * 128, num_idxs_reg=S2 * 128, elem_size=TW,
                        single_packet=False)

                # radial pipeline (edge-major)
                dif = ep.tile([128, S, 4], F32, tag="dif")
                nc.vector.tensor_tensor(dif[:], gr[:, :, 32:36], gc[:, :, 32:36],
                                        op=ALU.subtract)
                tt = ep.tile([128, S, 4], F32, tag="tt")
                nc.vector.tensor_scalar_mul(tt[:], dif[:], inv_l)
                gq = ep.tile([128, S, 4], F32, tag="gq")
                nc.vector.tensor_scalar(gq[:], tt[:], 0.5, None, op0=ALU.is_gt)
                nc.vector.tensor_scalar(tt[:], tt[:], -0.5, None, op0=ALU.is_lt)
                nc.vector.tensor_tensor(gq[:], gq[:], tt[:], op=ALU.subtract)
                # dif -= L * rnd
                nc.vector.scalar_tensor_tensor(
                    dif[:], in0=gq[:], scalar=-cfg.L, in1=dif[:],
                    op0=ALU.mult, op1=ALU.add)
                sq = ep.tile([128, S, 4], F32, tag="sq")
                nc.vector.tensor_tensor(sq[:], dif[:], dif[:], op=ALU.mult)
                rad = ep.tile([128, S], F32, tag="rad")
                nc.vector.tensor_reduce(
                    rad[:], sq[:], axis=mybir.AxisListType.X, op=ALU.add)
                srt = ep.tile([128, S], F32, tag="srt")
                nc.scalar.activation(srt[:], rad[:], ACTF.Sqrt, bias=eps16[:])
                rs = ep.tile([128, S], F32, tag="rs")
                nc.vector.reciprocal(rs[:], srt[:])
                cdif = ep.tile([128, S, 4], F32, tag="cdif")
                nc.vector.tensor_tensor(
                    cdif[:], dif[:],
                    AP(rs.tensor, rs[:].offset, [rs[:].ap[0], [1, S], [0, 4]]),
                    op=ALU.mult)

                # m1 = radial*w1c + A[row] + B[col] ; silu (bf16 out)
                m1 = ep.tile([128, S, H], F32, tag="m1")
                w1c = W(f"w1c{l}")
                nc.vector.tensor_tensor(
                    m1[:],
                    AP(wt.tensor, w1c.offset, [w1c.ap[0], [0, S], [1, H]]),
                    AP(rad.tensor, rad[:].offset, [rad[:].ap[0], [1, S], [0, H]]),
                    op=ALU.mult)
                nc.vector.tensor_tensor(m1[:], m1[:], gr[:, :, 0:32], op=ALU.add)
                nc.vector.tensor_tensor(m1[:], m1[:], gc[:, :, 0:32], op=ALU.add)
                m1s = ep.tile([128, S, H], BF16, tag="m1s")
                nc.scalar.activation(m1s[:], m1[:], ACTF.Silu)

                # one-hot [128, S, 128] (bf16 compare for 2x DVE rate)
                oh = ep.tile([128, S, 128], BF16, tag="oh")
                nc.vector.tensor_tensor(
                    oh[:],
                    AP(iotab.tensor, iotab[:].offset,
                       [iotab[:].ap[0], [0, S], [1, 128]]),
                    AP(rl16.tensor, rl16[:].offset,
                       [rl16[:].ap[0], [1, S], [0, 128]]),
                    op=ALU.is_equal)

                # MLP chain: full 4-subtile groups are processed in pairs
                # sharing each matmul/activation; a partial tail group (when
                # S % 4 != 0) runs alone at its own width so no garbage
                # partitions feed the block-diagonal weights.
                vals = tp.tile([128, S, 36], BF16, tag="vals")
                phps = psphi.tile([128, S], F32, tag="phps")
                nfull = S // 4          # groups with all 4 subtiles
                units = [list(t) for t in zip(range(0, nfull, 2),
                                              range(1, nfull, 2))]
                if nfull % 2:
                    units.append([nfull - 1])
                if S4 > nfull:
                    units.append([nfull])
                for gis in units:
                    t1 = pse.tile([128, 256], BF16, tag="pseb")
                    cofs = []
                    co = 0
                    for gi in gis:
                        p = min(4, S - gi * 4)
                        pk = 32 * p
                        cofs.append((gi, co, p, pk))
                        nc.tensor.transpose(
                            t1[0:pk, co:co + 128],
                            m1s[:, gi * 4:gi * 4 + p, :], identb[:])
                        co += 128
                    wid = co
                    pkw = cofs[0][3]  # uniform within a unit (128 or tail pk)
                    m1sT = tp.tile([128, 256], BF16, tag="m1sT")
                    nc.vector.tensor_copy(m1sT[0:pkw, 0:wid], t1[0:pkw, 0:wid])
                    mm = pse.tile([128, 256], F32, tag="pse")
                    nc.tensor.matmul(mm[0:pkw, 0:wid], lhsT=w2b[l][0:pkw, 0:pkw],
                                     rhs=m1sT[0:pkw, 0:wid])
                    mT = tp.tile([128, 256], BF16, tag="mT")
                    nc.scalar.activation(mT[0:pkw, 0:wid], mm[0:pkw, 0:wid],
                                         ACTF.Silu, bias=W(f"b2{l}")[0:pkw, :])
                    pp2 = pse.tile([128, 256], F32, tag="pse")
                    nc.tensor.matmul(pp2[0:pkw, 0:wid], lhsT=c1b[l][0:pkw, 0:pkw],
                                     rhs=mT[0:pkw, 0:wid])
                    p1T = tp.tile([128, 256], BF16, tag="p1T")
                    nc.scalar.activation(p1T[0:pkw, 0:wid], pp2[0:pkw, 0:wid],
                                         ACTF.Silu, bias=W(f"c1{l}")[0:pkw, :])
                    for gi, co, p, pk in cofs:
                        nc.tensor.matmul(
                            phps[:, gi * 4:gi * 4 + p],
                            lhsT=p1T[0:pk, co:co + 128],
                            rhs=c2b[l][0:pk, 0:p])
                        t2 = pse.tile([128, 128], BF16, tag="pseb")
                        nc.tensor.transpose(t2[:, 0:pk],
                                            mT[0:pk, co:co + 128],
                                            identb[0:pk, 0:pk])
                        nc.vector.tensor_copy(
                            vals[:, gi * 4:gi * 4 + p, 0:32],
                            t2[:, 0:pk].rearrange("e (j f) -> e j f", f=32))
                phi = ep.tile([128, S], F32, tag="phi")
                nc.scalar.activation(phi[:], phps[:], ACTF.Tanh,
                                     bias=W(f"c2{l}")[:, :])
                nc.vector.tensor_tensor(
                    vals[:, :, 32:35], cdif[:, :, 0:3],
                    AP(phi.tensor, phi[:].offset, [phi[:].ap[0], [1, S], [0, 3]]),
                    op=ALU.mult)

                agg = psagg.tile([35, 128], F32, tag="psagg")
                for s in range(S):
                    nc.tensor.matmul(agg[:], lhsT=vals[:, s, 0:35],
                                     rhs=oh[:, s, :],
                                     start=(s == 0), stop=(s == S - 1))
                nc.scalar.copy(aggT[0:35, b * 128:(b + 1) * 128], agg[:])
                tps = psn.tile([128, 4], F32, tag="psn")
                nc.tensor.transpose(tps[:, 0:3],
                                    aggT[32:35, b * 128:(b + 1) * 128],
                                    ident[32:35, 32:35])
                nc.scalar.copy(tsnm[:, b, 0:3], tps[:, 0:3])

            # ---- coord update ----
            nc.vector.tensor_tensor(
                tsnm[:, :, 0:3], tsnm[:, :, 0:3],
                AP(invc.tensor, invc[:].offset, [invc[:].ap[0], [1, BBLK], [0, 3]]),
                op=ALU.mult)
            nc.vector.tensor_tensor(coord[:, :, 0:3], coord[:, :, 0:3],
                                    tsnm[:, :, 0:3], op=ALU.add)

            # ---- node MLP + residual ----
            for o, w in CHUNKS:
                sl = slice(o, o + w)
                ps1 = psn.tile([32, 512], F32, tag="psn")
                nc.tensor.matmul(ps1[:, 0:w], lhsT=W(f"N1h{l}"), rhs=hT[:, sl],
                                 start=True, stop=False)
                nc.tensor.matmul(ps1[:, 0:w], lhsT=W(f"N1m{l}"), rhs=aggT[0:32, sl],
                                 start=False, stop=True)
                n1 = tp.tile([33, 512], F32, tag="n1")
                nc.vector.memset(n1[32:33, 0:w], 1.0)
                nc.scalar.activation(n1[0:32, 0:w], ps1[:, 0:w], ACTF.Silu)
                ps2 = psn.tile([32, 512], F32, tag="psn")
                nc.tensor.matmul(ps2[:, 0:w], lhsT=W(f"N2{l}"), rhs=n1[:, 0:w])
                nc.vector.tensor_tensor(hT[0:32, sl], hT[0:32, sl], ps2[:, 0:w],
                                        op=ALU.add)

        # ================= final =================
        for o, w in CHUNKS:
            sl = slice(o, o + w)
            ps = psn.tile([32, 512], F32, tag="psn")
            nc.tensor.matmul(ps[:, 0:w], lhsT=W("EOUT"), rhs=hT[:, sl])
            nc.scalar.copy(aggT[0:32, sl], ps[:, 0:w])
        psg = psphi.tile([32, G], F32, tag="phps")
        for b in range(BBLK):
            gohb = tp.tile([128, G], F32, tag="gohb")
            nc.sync.dma_start(gohb[:], ins["goh"][:, b * G:(b + 1) * G])
            t = psn.tile([128, 32], F32, tag="psn")
            nc.tensor.transpose(t[:, 0:32], aggT[0:32, b * 128:(b + 1) * 128],
                                ident[0:32, 0:32])
            onm = tp.tile([128, 32], F32, tag="n1")
            nc.scalar.copy(onm[:], t[:, 0:32])
            nc.tensor.matmul(psg[:], lhsT=onm[:], rhs=gohb[:],
                             start=(b == 0), stop=(b == BBLK - 1))
        gsb = wp.tile([32, G], F32, tag="gsb")
        nc.scalar.copy(gsb[:], psg[:])
        nc.sync.dma_start(gsin[:], gsb[:])
        nc.gpsimd.collective_compute(
            "AllReduce", ALU.add, replica_groups=[list(range(NC))],
            ins=[gsin[:]], outs=[gsout[:]],
        )
        hg = wp.tile([33, G], F32, tag="hg")
        nc.vector.memset(hg[32:33, :], 1.0)
        nc.sync.dma_start(hg[0:32, :], gsout[:])
        pm = psn.tile([32, G], F32, tag="psn")
        nc.tensor.matmul(pm[:], lhsT=W("ZMU"), rhs=hg[:])
        msb = wp.tile([32, G], F32, tag="msb")
        nc.scalar.copy(msb[:], pm[:])
        nc.sync.dma_start(mu_o[:], msb[:])
        ps2 = psn.tile([32, G], F32, tag="psn")
        nc.tensor.matmul(ps2[:], lhsT=W("ZSIG"), rhs=hg[:])
        # softplus(x) = max(x,0) + ln(1 + exp(-|x|))
        zsb = wp.tile([32, G], F32, tag="zsb")
        nc.scalar.copy(zsb[:], ps2[:])
        axp = wp.tile([32, G], F32, tag="axp")
        nc.scalar.activation(axp[:], zsb[:], ACTF.Abs)
        nc.scalar.activation(axp[:], axp[:], ACTF.Exp, scale=-1.0)
        nc.vector.tensor_scalar_add(axp[:], axp[:], 1.0)
        nc.scalar.activation(axp[:], axp[:], ACTF.Ln)
        ssb = wp.tile([32, G], F32, tag="ssb")
        nc.vector.tensor_scalar(ssb[:], zsb[:], 0.0, None, op0=ALU.max)
        nc.vector.tensor_tensor(ssb[:], ssb[:], axp[:], op=ALU.add)
        nc.sync.dma_start(sig_o[:], ssb[:])


# ----------------------------------------------------------------------
# kernel() entry point: full inputs -> full outputs, compiled + run on the
# 8 NeuronCores via run_bass_kernel_spmd (SPMD, one NEFF).
# ----------------------------------------------------------------------
import os

_CACHE = {}


def kernel(**inputs):
    import numpy as np
    from concourse import bacc
    from concourse.tile import TileContext
    from concourse.bass_utils import run_bass_kernel_spmd

    N = int(inputs["x"].shape[0])
    E = int(inputs["edge_index"].shape[1])
    G = 64
    lval = float(np.asarray(inputs["l"]).reshape(-1)[0])

    cfg = Cfg(N=N, E=E, G=G, L=lval)
    per_core = preprocess(inputs, cfg)
    specs = input_specs(cfg)

    key = (N, E, G, cfg.S1, cfg.S2, cfg.WC)
    if key in _CACHE:
        nc = _CACHE[key]
    else:
        nc = bacc.Bacc("TRN2", target_bir_lowering=False, debug=False,
                       num_devices=NC)
        ins = {}
        for k, (shape, dt) in specs.items():
            mdt = {np.int16: mybir.dt.int16, np.float32: mybir.dt.float32}[dt]
            ins[k] = nc.dram_tensor(k, list(shape), mdt, kind="ExternalInput").ap()
        outs = {
            "mu": nc.dram_tensor("mu", [32, G], F32, kind="ExternalOutput").ap(),
            "sig": nc.dram_tensor("sig", [32, G], F32, kind="ExternalOutput").ap(),
        }
        with TileContext(nc) as tc:
            build(tc, outs, ins, cfg)
        nc.compile()
        _CACHE[key] = nc

    in_maps = []
    for c in range(NC):
        m = {}
        for k, (shape, dt) in specs.items():
            a = np.ascontiguousarray(per_core[c][k]).astype(dt)
            assert a.shape == tuple(shape), (k, a.shape, shape)
            m[k] = a
        in_maps.append(m)

    res = run_bass_kernel_spmd(nc, in_maps, core_ids=list(range(NC)))
    r0 = res.results[0]
    mu = np.ascontiguousarray(r0["mu"].T)
    sigma = np.ascontiguousarray(r0["sig"].T)

    iters = int(os.environ.get("EGNN_BENCH", "0"))
    if iters > 0:
        _bench_pjrt(nc, in_maps, NC, iters)
    return mu, sigma


def _bench_pjrt(nc, in_maps, n_cores, iters):
    """Measure per-execution device time by pipelining batches of
    executions through PJRT and fitting the slope (amortizes the large
    fixed dispatch overhead of the remote device path)."""
    import time
    import numpy as np
    import jax
    from jax.sharding import Mesh, PartitionSpec, NamedSharding
    from jax.experimental.shard_map import shard_map
    import concourse.mybir as mybir
    from concourse import bass2jax
    from concourse.bass2jax import _bass_exec_p, partition_id_tensor

    bass2jax.install_neuronx_cc_hook()
    partition_name = (nc.partition_id_tensor.name
                      if nc.partition_id_tensor else None)
    in_names, out_names, out_avals, zero_outs = [], [], [], []
    for alloc in nc.m.functions[0].allocations:
        if not isinstance(alloc, mybir.MemoryLocationSet):
            continue
        name = alloc.memorylocations[0].name
        if alloc.kind == "ExternalInput":
            if name != partition_name:
                in_names.append(name)
        elif alloc.kind == "ExternalOutput":
            shape = tuple(alloc.tensor_shape)
            dtype = mybir.dt.np(alloc.dtype)
            out_names.append(name)
            out_avals.append(jax.core.ShapedArray(shape, dtype))
            zero_outs.append(np.zeros(shape, dtype))
    n_params = len(in_names)
    all_in_names = list(in_names) + list(out_names)
    if partition_name is not None:
        all_in_names.append(partition_name)

    def _body(*args):
        operands = list(args)
        if partition_name is not None:
            operands.append(partition_id_tensor())
        outs = _bass_exec_p.bind(
            *operands, out_avals=tuple(out_avals),
            in_names=tuple(all_in_names), out_names=tuple(out_names),
            lowering_input_output_aliases=(),
            sim_require_finite=True, sim_require_nnan=True, nc=nc)
        return tuple(outs)

    devices = jax.devices()[:n_cores]
    mesh = Mesh(np.asarray(devices), ("core",))
    spec = PartitionSpec("core")
    fn = jax.jit(shard_map(_body, mesh=mesh,
                           in_specs=(spec,) * (n_params + len(out_names)),
                           out_specs=(spec,) * len(out_names),
                           check_rep=False), keep_unused=True)
    sh = NamedSharding(mesh, spec)
    concat_in = [
        jax.device_put(
            np.concatenate([np.asarray(in_maps[c][nm]) for c in range(n_cores)],
                           axis=0), sh)
        for nm in in_names
    ]
    concat_zero = [
        jax.device_put(np.zeros((n_cores * z.shape[0], *z.shape[1:]), z.dtype), sh)
        for z in zero_outs
    ]
    outs = fn(*concat_in, *concat_zero)
    jax.block_until_ready(outs)  # warm compile + first exec

    def run_batch(k):
        t0 = time.perf_counter()
        outs = [fn(*concat_in, *concat_zero) for _ in range(k)]
        jax.block_until_ready(outs)
        return time.perf_counter() - t0

    k_lo, k_hi = 4, 4 + 16 * max(1, iters)
    t_lo = min(run_batch(k_lo) for _ in range(2))
    t_hi = min(run_batch(k_hi) for _ in range(2))
    per_exec = (t_hi - t_lo) / (k_hi - k_lo)
    print(f"batch timing: T({k_lo})={t_lo*1e3:.1f}ms T({k_hi})={t_hi*1e3:.1f}ms")
    print(f"HW exec time: {int(per_exec * 1e9)} ns")


# revision 16
# speedup vs baseline: 1.5496x; 1.5496x over previous
"""EGNN encoder kernel for Trainium2 (Bass/Tile), 8-core SPMD.

Strategy:
  - Edge-parallel: core c owns destination rows [c*ROWN, (c+1)*ROWN).
  - Edges sorted by row globally; per core, edges grouped into 128-row
    blocks; within a block edges are sorted by col and split at col SPLIT
    so dma_gather's int16 indices work in two passes.
  - Per layer, node tables tabA=[h@W1a+b1 | coord] (own rows, local) and
    tabB=[h@W1b | coord] (all rows) live in DRAM with 64-float rows
    (256B, dma_gather elem constraint).  tabB content is exchanged via a
    compact [ROWN,36] AllGather (ping-pong input buffers to avoid
    straggler overwrite races) and repacked locally into the 256B-row
    gather table.
  - Edge MLP runs feat-major on PE via batched transposes (2 groups of 4
    subtiles share each matmul); segment-sum via one-hot matmuls
    accumulating into PSUM per 128-row block.
"""

import math
import os
from contextlib import ExitStack
from dataclasses import dataclass, field

PROBE_NO_COLL = bool(int(os.environ.get("EGNN_NO_COLL", "0")))
PROBE_NO_GATHER = bool(int(os.environ.get("EGNN_NO_GATHER", "0")))

import numpy as np

import concourse.bass as bass
import concourse.tile as tile
from concourse import mybir
from concourse.bass import AP
from concourse.masks import make_identity

F32 = mybir.dt.float32
BF16 = mybir.dt.bfloat16
I32 = mybir.dt.int32
I16 = mybir.dt.int16
ALU = mybir.AluOpType
ACTF = mybir.ActivationFunctionType

NC = 8        # cores
H = 32        # hidden
NL = 4        # layers
TW = 64       # table row width (floats) = 256B for dma_gather
CW = 36       # compact row width (h 32 + coord 4)


@dataclass
class Cfg:
    N: int
    E: int
    G: int
    L: float = 10.0
    EPS: float = 1e-8
    SPLIT: int = 32768  # int16 index limit for col gathers
    # derived
    ROWN: int = 0
    BBLK: int = 0
    RPAD: int = 0
    NPAD: int = 0
    S1: int = 0
    S2: int = 0
    S: int = 0
    CHUNKS: list = field(default_factory=list)
    wslots: dict = field(default_factory=dict)
    WC: int = 0

    def derive_static(self):
        self.ROWN = self.N // NC
        self.BBLK = (self.ROWN + 127) // 128
        self.RPAD = self.BBLK * 128
        self.NPAD = ((self.N + 127) // 128) * 128
        # dense node-phase chunks (512 then remainder)
        self.CHUNKS = []
        o = 0
        while o < self.RPAD:
            w = min(512, self.RPAD - o)
            self.CHUNKS.append((o, w))
            o += w


# ---------------------------------------------------------------- host pre

def preprocess(inp, cfg: Cfg):
    """Sort/shard edges, build per-core index arrays and weight pack."""
    cfg.derive_static()
    N, E, G = cfg.N, cfg.E, cfg.G
    ROWN, BBLK = cfg.ROWN, cfg.BBLK

    row = np.asarray(inp["edge_index"][0]).astype(np.int64)
    col = np.asarray(inp["edge_index"][1]).astype(np.int64)
    order = np.argsort(row, kind="stable")
    rs, cs = row[order], col[order]

    # per-core, per-block edge lists (block = 128 dest rows), col-sorted
    core_edges = []  # [core][block] -> (r_loc, c_gl, nlo) col-split
    S1 = S2 = 0
    for c in range(NC):
        lo = np.searchsorted(rs, c * ROWN)
        hi = np.searchsorted(rs, (c + 1) * ROWN)
        r_loc = rs[lo:hi] - c * ROWN
        c_gl = cs[lo:hi]
        blk = r_loc // 128
        blocks = []
        for b in range(BBLK):
            blo = np.searchsorted(blk, b)
            bhi = np.searchsorted(blk, b + 1)
            rb, cb = r_loc[blo:bhi], c_gl[blo:bhi]
            o2 = np.argsort(cb, kind="stable")
            rb, cb = rb[o2], cb[o2]
            nlo = int(np.searchsorted(cb, cfg.SPLIT))
            nhi = len(cb) - nlo
            S1 = max(S1, (nlo + 127) // 128)
            S2 = max(S2, (nhi + 127) // 128)
            blocks.append((rb, cb, nlo))
        core_edges.append(blocks)
    S1 = max(S1, 1)
    cfg.S1, cfg.S2, cfg.S = S1, S2, S1 + S2
    S = cfg.S

    def wrap16(a_i16):
        # [BBLK, S*128] entry i -> partition i%16 col i//16, replicated x8
        x = a_i16.reshape(BBLK, S * 128 // 16, 16)  # [b, col, p]
        x = x.transpose(2, 0, 1).reshape(16, BBLK * S * 8)
        return np.tile(x, (8, 1)).astype(np.int16)  # [128, BBLK*S*8]

    per_core = []
    batch = np.asarray(inp["batch"]).astype(np.int64)
    gcnt = np.bincount(batch, minlength=G).astype(np.float64)
    invg = (1.0 / np.maximum(gcnt, 1.0)).astype(np.float32)
    pos = np.asarray(inp["pos"]).astype(np.float32)
    x_in = np.asarray(inp["x"]).astype(np.float32).reshape(-1)

    for c in range(NC):
        idxr = np.zeros((BBLK, S * 128), np.int16)
        idxc = np.zeros((BBLK, S * 128), np.int16)
        rloc = np.full((BBLK, S * 128), -1.0, np.float32)
        cnt = np.zeros(cfg.RPAD, np.float64)
        for b in range(BBLK):
            rb, cb, nlo = core_edges[c][b]
            np.add.at(cnt, rb, 1.0)
            # lo edges at entries [0, nlo), hi edges at [S1*128, S1*128+nhi)
            idxr[b, :nlo] = rb[:nlo]
            idxc[b, :nlo] = cb[:nlo]
            rloc[b, :nlo] = (rb[:nlo] - 128 * b).astype(np.float32)
            nhi = len(rb) - nlo
            o = S1 * 128
            idxr[b, o:o + nhi] = rb[nlo:]
            idxc[b, o:o + nhi] = cb[nlo:] - cfg.SPLIT
            rloc[b, o:o + nhi] = (rb[nlo:] - 128 * b).astype(np.float32)
        d = {}
        if bool(int(os.environ.get("EGNN_ZERO_IDX", "0"))):
            idxr = np.zeros_like(idxr)
            idxc = np.zeros_like(idxc)
        wr, wc = wrap16(idxr), wrap16(idxc)
        # merged stream: per block [idxr S*8 | idxc S*8]
        m = np.zeros((128, BBLK * S * 16), np.int16)
        for b in range(BBLK):
            m[:, b * S * 16:b * S * 16 + S * 8] = wr[:, b * S * 8:(b + 1) * S * 8]
            m[:, b * S * 16 + S * 8:(b + 1) * S * 16] = wc[:, b * S * 8:(b + 1) * S * 8]
        d["idxrc"] = m
        # rloc layout: [128, BBLK*S], entry e=(sub*128+p) -> [p, b*S+sub]
        d["rloc"] = (
            rloc.reshape(BBLK, S, 128).transpose(2, 0, 1).reshape(128, BBLK * S)
        ).astype(np.float32).copy()
        inv = (1.0 / np.maximum(cnt, 1.0)).astype(np.float32)
        d["invc"] = inv.reshape(BBLK, 128).T.copy()  # [128, BBLK]
        # pos node-major [128, BBLK*4] (4th component stays 0)
        p4 = np.zeros((cfg.RPAD, 4), np.float32)
        p4[:ROWN, :3] = pos[c * ROWN:(c + 1) * ROWN]
        d["posnm"] = p4.reshape(BBLK, 128, 4).transpose(1, 0, 2).reshape(128, BBLK * 4).copy()
        # x2 [2, RPAD] rows: x, ones
        x2 = np.zeros((2, cfg.RPAD), np.float32)
        x2[0, :ROWN] = x_in[c * ROWN:(c + 1) * ROWN]
        x2[1, :] = 1.0
        d["x2"] = x2
        # weighted graph one-hot node-major [128, BBLK*G]
        goh = np.zeros((cfg.RPAD, G), np.float32)
        bloc = batch[c * ROWN:(c + 1) * ROWN]
        goh[np.arange(ROWN), bloc] = invg[bloc]
        d["goh"] = goh.reshape(BBLK, 128, G).transpose(1, 0, 2).reshape(128, BBLK * G).copy()
        per_core.append(d)

    # ---- weight pack (shared across cores) --------------------------------
    slots = {}
    colp = [0]
    wpack_parts = []

    def blkdiag(w, n=4):
        k, m = w.shape
        out = np.zeros((n * k, n * m), np.float32)
        for i in range(n):
            out[i * k:(i + 1) * k, i * m:(i + 1) * m] = w
        return out

    def add(name, arr):
        arr = np.asarray(arr, np.float32)
        assert arr.ndim == 2 and arr.shape[0] <= 128
        c0 = colp[0]
        slots[name] = (arr.shape[0], c0, arr.shape[1])
        colp[0] += arr.shape[1]
        wpack_parts.append((c0, arr))

    g = lambda k: np.asarray(inp[k], np.float32)
    add("EMB", np.vstack([g("emb_in_w"), g("emb_in_b")[None, :]]))
    for l in range(NL):
        w1 = g("edge_w1")[l]; b1 = g("edge_b1")[l]
        ab = np.zeros((33, 64), np.float32)
        ab[:32, :32] = w1[0:32]; ab[32, :32] = b1
        ab[:32, 32:] = w1[32:64]
        add(f"AB{l}", ab)
        add(f"w1c{l}", np.tile(w1[64:65, :], (128, 1)))
        add(f"W2{l}", blkdiag(g("edge_w2")[l]))
        add(f"b2{l}", np.tile(g("edge_b2")[l], 4)[:, None])
        add(f"C1{l}", blkdiag(g("coord_w1")[l]))
        add(f"c1{l}", np.tile(g("coord_b1")[l], 4)[:, None])
        add(f"C2{l}", blkdiag(g("coord_w2")[l]))  # [128, 4]
        add(f"c2{l}", np.full((128, 1), float(g("coord_b2")[l][0]), np.float32))
        add(f"N1h{l}", np.vstack([g("node_w1")[l][0:32], g("node_b1")[l][None, :]]))
        add(f"N1m{l}", g("node_w1")[l][32:64])
        add(f"N2{l}", np.vstack([g("node_w2")[l], g("node_b2")[l][None, :]]))
    add("EOUT", np.vstack([g("emb_out_w"), g("emb_out_b")[None, :]]))
    add("ZMU", np.vstack([g("zmu_w"), g("zmu_b")[None, :]]))
    add("ZSIG", np.vstack([g("zsig_w"), g("zsig_b")[None, :]]))
    cfg.WC = colp[0]
    cfg.wslots = slots
    wpack = np.zeros((128, cfg.WC), np.float32)
    for c0, arr in wpack_parts:
        wpack[: arr.shape[0], c0:c0 + arr.shape[1]] = arr

    for d in per_core:
        d["wpack"] = wpack
    return per_core


def input_specs(cfg: Cfg):
    """name -> (shape, np dtype) for per-core device inputs."""
    S, BBLK, RPAD, G = cfg.S, cfg.BBLK, cfg.RPAD, cfg.G
    return {
        "idxrc": ((128, BBLK * S * 16), np.int16),
        "rloc": ((128, BBLK * S), np.float32),
        "invc": ((128, BBLK), np.float32),
        "posnm": ((128, BBLK * 4), np.float32),
        "x2": ((2, RPAD), np.float32),
        "goh": ((128, BBLK * G), np.float32),
        "wpack": ((128, cfg.WC), np.float32),
    }


# ---------------------------------------------------------------- builder

def build(tc, outs, ins, cfg: Cfg):
    nc = tc.nc
    S, S1, S2 = cfg.S, cfg.S1, cfg.S2
    BBLK, RPAD, NPAD, G = cfg.BBLK, cfg.RPAD, cfg.NPAD, cfg.G
    N, ROWN = cfg.N, cfg.ROWN
    CHUNKS = cfg.CHUNKS
    S4 = (S + 3) // 4
    inv_l = 1.0 / cfg.L

    mu_o, sig_o = outs["mu"], outs["sig"]

    tabA = nc.dram_tensor("tabA", [RPAD, TW], F32, kind="Internal")
    tabBoc = [nc.dram_tensor(f"tabBoc{p}", [RPAD, CW], F32, kind="Internal")
              for p in range(2)]
    tabBc = nc.dram_tensor("tabBc", [N, CW], F32, kind="Internal",
                           addr_space="Shared")
    tabB = nc.dram_tensor("tabB", [NPAD, TW], F32, kind="Internal")
    gsin = nc.dram_tensor("gsin", [32, G], F32, kind="Internal")
    gsout = nc.dram_tensor("gsout", [32, G], F32, kind="Internal",
                           addr_space="Shared")

    ctx = ExitStack()
    with ctx:
        wp = ctx.enter_context(tc.tile_pool(name="wp", bufs=1))
        ep = ctx.enter_context(tc.tile_pool(name="ep", bufs=2))
        tp = ctx.enter_context(tc.tile_pool(name="tp", bufs=2))
        pse = ctx.enter_context(tc.tile_pool(name="pse", bufs=2, space="PSUM"))
        psphi = ctx.enter_context(tc.tile_pool(name="psphi", bufs=1, space="PSUM"))
        psagg = ctx.enter_context(tc.tile_pool(name="psagg", bufs=1, space="PSUM"))
        psn = ctx.enter_context(tc.tile_pool(name="psn", bufs=2, space="PSUM"))

        # ---------------- persistent state ----------------
        wt = wp.tile([128, cfg.WC], F32, tag="wt")
        nc.sync.dma_start(wt[:], ins["wpack"][:])

        def W(name):
            p, c0, w = cfg.wslots[name]
            return wt[0:p, c0:c0 + w]

        ident = wp.tile([128, 128], F32, tag="ident")
        make_identity(nc, ident[:])
        identb = wp.tile([128, 128], BF16, tag="identb")
        nc.vector.tensor_copy(identb[:], ident[:])
        w2b, c1b, c2b = [], [], []
        for l in range(NL):
            w2b.append(wp.tile([128, 128], BF16, tag=f"w2b{l}", name=f"w2b{l}"))
            c1b.append(wp.tile([128, 128], BF16, tag=f"c1b{l}", name=f"c1b{l}"))
            c2b.append(wp.tile([128, 4], BF16, tag=f"c2b{l}", name=f"c2b{l}"))
        iota = wp.tile([128, 128], I32, tag="iotai")
        nc.gpsimd.iota(iota[:], pattern=[[1, 128]], base=0, channel_multiplier=0)
        iotab = wp.tile([128, 128], BF16, tag="iotab")
        nc.vector.tensor_copy(iotab[:], iota[:])

        for l in range(NL):
            nc.vector.tensor_copy(w2b[l][:], W(f"W2{l}"))
            nc.vector.tensor_copy(c1b[l][:], W(f"C1{l}"))
            nc.vector.tensor_copy(c2b[l][:], W(f"C2{l}"))

        invc = wp.tile([128, BBLK], F32, tag="invc")
        nc.sync.dma_start(invc[:], ins["invc"][:])

        hT = wp.tile([33, RPAD], F32, tag="hT")
        nc.vector.memset(hT[32:33, :], 1.0)
        coord = wp.tile([128, BBLK, 4], F32, tag="coord")
        nc.sync.dma_start(coord[:], ins["posnm"][:].rearrange("p (b c) -> p b c", c=4))
        aggT = wp.tile([36, RPAD], F32, tag="aggT")
        tsnm = wp.tile([128, BBLK, 4], F32, tag="tsnm")

        # zero the pad regions of the DRAM tables once: columns CW:TW of
        # every row, plus tabB rows N:NPAD entirely.
        zt = wp.tile([128, TW], F32, tag="zt")
        nc.vector.memset(zt[:], 0.0)
        eps16 = wp.tile([128, 1], F32, tag="eps16")
        nc.vector.memset(eps16[:], 1e-16)

        def bc_mid(ap_, count):
            # [P, W] -> [P, count, W] with stride-0 middle dim
            return AP(ap_.tensor, ap_.offset, [ap_.ap[0], [0, count], ap_.ap[1]])

        nc.sync.dma_start(
            tabA[:].rearrange("(b p) c -> p b c", p=128)[:, :, CW:TW],
            bc_mid(zt[:, 0:TW - CW], RPAD // 128))
        nc.sync.dma_start(
            tabB[:].rearrange("(b p) c -> p b c", p=128)[:, :, CW:TW],
            bc_mid(zt[:, 0:TW - CW], NPAD // 128))
        if NPAD > N:
            nc.sync.dma_start(tabB[N:NPAD, 0:CW], zt[0:NPAD - N, 0:CW])

        # h = emb_in(x): hT[0:32] = EMB.T @ x2
        for o, w in CHUNKS:
            sl = slice(o, o + w)
            x2c = tp.tile([2, 512], F32, tag="x2c")
            nc.sync.dma_start(x2c[:, 0:w], ins["x2"][:, sl])
            ps = psn.tile([32, 512], F32, tag="psn")
            nc.tensor.matmul(ps[:, 0:w], lhsT=W("EMB"), rhs=x2c[:, 0:w])
            nc.scalar.copy(hT[0:32, sl], ps[:, 0:w])

        # ================= layers =================
        for l in range(NL):
            pp_ = l % 2
            # ---- write tabA (local) and tabBoc (collective input) ----
            for b in range(BBLK):
                bsl = slice(b * 128, (b + 1) * 128)
                ps = psn.tile([128, 64], F32, tag="psn")
                nc.tensor.matmul(ps[:], lhsT=hT[:, bsl], rhs=W(f"AB{l}"))
                tba = tp.tile([128, CW], F32, tag="tba")
                nc.scalar.copy(tba[:, 0:32], ps[:, 0:32])
                nc.vector.tensor_copy(tba[:, 32:36], coord[:, b, 0:4])
                nc.sync.dma_start(
                    tabA[:].rearrange("(b p) c -> p b c", p=128)[:, b, 0:CW],
                    tba[:])
                tbb = tp.tile([128, CW], F32, tag="tbb")
                nc.scalar.copy(tbb[:, 0:32], ps[:, 32:64])
                nc.vector.tensor_copy(tbb[:, 32:36], coord[:, b, 0:4])
                nc.sync.dma_start(
                    tabBoc[pp_][:].rearrange("(b p) c -> p b c", p=128)[:, b, :],
                    tbb[:])
            if not PROBE_NO_COLL:
                nc.gpsimd.collective_compute(
                    "AllGather", ALU.bypass,
                    replica_groups=[list(range(NC))],
                    ins=[tabBoc[pp_][0:ROWN, :]],
                    outs=[tabBc[0:N, :]],
                )
            # repack compact rows into the 256B-row gather table
            nc.sync.dma_start(tabB[0:N, 0:CW], tabBc[0:N, :])

            # ---- edge phase ----
            for b in range(BBLK):
                # stream indices for this block
                ixrc = ep.tile([128, S * 16], I16, tag="ixrc")
                nc.sync.dma_start(ixrc[:], ins["idxrc"][:, b * S * 16:(b + 1) * S * 16])
                rlocb = ep.tile([128, S], F32, tag="rlocb")
                nc.sync.dma_start(rlocb[:], ins["rloc"][:, b * S:(b + 1) * S])
                rl16 = ep.tile([128, S], BF16, tag="rl16")
                nc.vector.tensor_copy(rl16[:], rlocb[:])

                gr = ep.tile([128, S, TW], F32, tag="gr")
                if not PROBE_NO_GATHER:
                  nc.gpsimd.dma_gather(
                    out_ap=gr[:], in_ap=tabA[:], idxs_ap=ixrc[:, 0:S * 8],
                    num_idxs=S * 128, num_idxs_reg=S * 128, elem_size=TW,
                    single_packet=False)
                gc = ep.tile([128, S, TW], F32, tag="gc")
                if not PROBE_NO_GATHER:
                  nc.gpsimd.dma_gather(
                    out_ap=gc[:, 0:S1, :], in_ap=tabB[0:min(cfg.SPLIT, NPAD), :],
                    idxs_ap=ixrc[:, S * 8:S * 8 + S1 * 8],
                    num_idxs=S1 * 128, num_idxs_reg=S1 * 128, elem_size=TW,
                    single_packet=False)
                  if S2 > 0:
                    nc.gpsimd.dma_gather(
                        out_ap=gc[:, S1:S, :], in_ap=tabB[cfg.SPLIT:NPAD, :],
                        idxs_ap=ixrc[:, S * 8 + S1 * 8:S * 16],
                        num_idxs=S2 * 128, num_idxs_reg=S2 * 128, elem_size=TW,
                        single_packet=False)

                # radial pipeline (edge-major)
                dif = ep.tile([128, S, 4], F32, tag="dif")
                nc.vector.tensor_tensor(dif[:], gr[:, :, 32:36], gc[:, :, 32:36],
                                        op=ALU.subtract)
                tt = ep.tile([128, S, 4], F32, tag="tt")
                nc.vector.tensor_scalar_mul(tt[:], dif[:], inv_l)
                gq = ep.tile([128, S, 4], F32, tag="gq")
                nc.vector.tensor_scalar(gq[:], tt[:], 0.5, None, op0=ALU.is_gt)
                nc.vector.tensor_scalar(tt[:], tt[:], -0.5, None, op0=ALU.is_lt)
                nc.vector.tensor_tensor(gq[:], gq[:], tt[:], op=ALU.subtract)
                # dif -= L * rnd
                nc.vector.scalar_tensor_tensor(
                    dif[:], in0=gq[:], scalar=-cfg.L, in1=dif[:],
                    op0=ALU.mult, op1=ALU.add)
                sq = ep.tile([128, S, 4], F32, tag="sq")
                nc.vector.tensor_tensor(sq[:], dif[:], dif[:], op=ALU.mult)
                rad = ep.tile([128, S], F32, tag="rad")
                nc.vector.tensor_reduce(
                    rad[:], sq[:], axis=mybir.AxisListType.X, op=ALU.add)
                srt = ep.tile([128, S], F32, tag="srt")
                nc.scalar.activation(srt[:], rad[:], ACTF.Sqrt, bias=eps16[:])
                rs = ep.tile([128, S], F32, tag="rs")
                nc.vector.reciprocal(rs[:], srt[:])
                cdif = ep.tile([128, S, 4], F32, tag="cdif")
                nc.vector.tensor_tensor(
                    cdif[:], dif[:],
                    AP(rs.tensor, rs[:].offset, [rs[:].ap[0], [1, S], [0, 4]]),
                    op=ALU.mult)

                # m1 = radial*w1c + A[row] + B[col] ; silu (bf16 out)
                m1 = ep.tile([128, S, H], F32, tag="m1")
                w1c = W(f"w1c{l}")
                nc.vector.tensor_tensor(
                    m1[:],
                    AP(wt.tensor, w1c.offset, [w1c.ap[0], [0, S], [1, H]]),
                    AP(rad.tensor, rad[:].offset, [rad[:].ap[0], [1, S], [0, H]]),
                    op=ALU.mult)
                nc.vector.tensor_tensor(m1[:], m1[:], gr[:, :, 0:32], op=ALU.add)
                nc.vector.tensor_tensor(m1[:], m1[:], gc[:, :, 0:32], op=ALU.add)
                m1s = ep.tile([128, S, H], BF16, tag="m1s")
                nc.scalar.activation(m1s[:], m1[:], ACTF.Silu)

                # one-hot [128, S, 128] (bf16 compare for 2x DVE rate)
                oh = ep.tile([128, S, 128], BF16, tag="oh")
                nc.vector.tensor_tensor(
                    oh[:],
                    AP(iotab.tensor, iotab[:].offset,
                       [iotab[:].ap[0], [0, S], [1, 128]]),
                    AP(rl16.tensor, rl16[:].offset,
                       [rl16[:].ap[0], [1, S], [0, 128]]),
                    op=ALU.is_equal)

                # MLP chain: full 4-subtile groups are processed in pairs
                # sharing each matmul/activation; a partial tail group (when
                # S % 4 != 0) runs alone at its own width so no garbage
                # partitions feed the block-diagonal weights.
                vals = tp.tile([128, S, 36], BF16, tag="vals")
                phps = psphi.tile([128, S], F32, tag="phps")
                nfull = S // 4          # groups with all 4 subtiles
                units = [list(t) for t in zip(range(0, nfull, 2),
                                              range(1, nfull, 2))]
                if nfull % 2:
                    units.append([nfull - 1])
                if S4 > nfull:
                    units.append([nfull])
                for gis in units:
                    t1 = pse.tile([128, 256], BF16, tag="pseb")
                    cofs = []
                    co = 0
                    for gi in gis:
                        p = min(4, S - gi * 4)
                        pk = 32 * p
                        cofs.append((gi, co, p, pk))
                        nc.tensor.transpose(
                            t1[0:pk, co:co + 128],
                            m1s[:, gi * 4:gi * 4 + p, :], identb[:])
                        co += 128
                    wid = co
                    pkw = cofs[0][3]  # uniform within a unit (128 or tail pk)
                    m1sT = tp.tile([128, 256], BF16, tag="m1sT")
                    nc.vector.tensor_copy(m1sT[0:pkw, 0:wid], t1[0:pkw, 0:wid])
                    mm = pse.tile([128, 256], F32, tag="pse")
                    nc.tensor.matmul(mm[0:pkw, 0:wid], lhsT=w2b[l][0:pkw, 0:pkw],
                                     rhs=m1sT[0:pkw, 0:wid])
                    mT = tp.tile([128, 256], BF16, tag="mT")
                    nc.scalar.activation(mT[0:pkw, 0:wid], mm[0:pkw, 0:wid],
                                         ACTF.Silu, bias=W(f"b2{l}")[0:pkw, :])
                    pp2 = pse.tile([128, 256], F32, tag="pse")
                    nc.tensor.matmul(pp2[0:pkw, 0:wid], lhsT=c1b[l][0:pkw, 0:pkw],
                                     rhs=mT[0:pkw, 0:wid])
                    p1T = tp.tile([128, 256], BF16, tag="p1T")
                    nc.scalar.activation(p1T[0:pkw, 0:wid], pp2[0:pkw, 0:wid],
                                         ACTF.Silu, bias=W(f"c1{l}")[0:pkw, :])
                    for gi, co, p, pk in cofs:
                        nc.tensor.matmul(
                            phps[:, gi * 4:gi * 4 + p],
                            lhsT=p1T[0:pk, co:co + 128],
                            rhs=c2b[l][0:pk, 0:p])
                        t2 = pse.tile([128, 128], BF16, tag="pseb")
                        nc.tensor.transpose(t2[:, 0:pk],
                                            mT[0:pk, co:co + 128],
                                            identb[0:pk, 0:pk])
                        nc.vector.tensor_copy(
                            vals[:, gi * 4:gi * 4 + p, 0:32],
                            t2[:, 0:pk].rearrange("e (j f) -> e j f", f=32))
                phi = ep.tile([128, S], F32, tag="phi")
                nc.scalar.activation(phi[:], phps[:], ACTF.Tanh,
                                     bias=W(f"c2{l}")[:, :])
                nc.vector.tensor_tensor(
                    vals[:, :, 32:35], cdif[:, :, 0:3],
                    AP(phi.tensor, phi[:].offset, [phi[:].ap[0], [1, S], [0, 3]]),
                    op=ALU.mult)

                agg = psagg.tile([35, 128], F32, tag="psagg")
                for s in range(S):
                    nc.tensor.matmul(agg[:], lhsT=vals[:, s, 0:35],
                                     rhs=oh[:, s, :],
                                     start=(s == 0), stop=(s == S - 1))
                nc.scalar.copy(aggT[0:35, b * 128:(b + 1) * 128], agg[:])
                tps = psn.tile([128, 4], F32, tag="psn")
                nc.tensor.transpose(tps[:, 0:3],
                                    aggT[32:35, b * 128:(b + 1) * 128],
                                    ident[32:35, 32:35])
                nc.scalar.copy(tsnm[:, b, 0:3], tps[:, 0:3])

            # ---- coord update ----
            nc.vector.tensor_tensor(
                tsnm[:, :, 0:3], tsnm[:, :, 0:3],
                AP(invc.tensor, invc[:].offset, [invc[:].ap[0], [1, BBLK], [0, 3]]),
                op=ALU.mult)
            nc.vector.tensor_tensor(coord[:, :, 0:3], coord[:, :, 0:3],
                                    tsnm[:, :, 0:3], op=ALU.add)

            # ---- node MLP + residual ----
            for o, w in CHUNKS:
                sl = slice(o, o + w)
                ps1 = psn.tile([32, 512], F32, tag="psn")
                nc.tensor.matmul(ps1[:, 0:w], lhsT=W(f"N1h{l}"), rhs=hT[:, sl],
                                 start=True, stop=False)
                nc.tensor.matmul(ps1[:, 0:w], lhsT=W(f"N1m{l}"), rhs=aggT[0:32, sl],
                                 start=False, stop=True)
                n1 = tp.tile([33, 512], F32, tag="n1")
                nc.vector.memset(n1[32:33, 0:w], 1.0)
                nc.scalar.activation(n1[0:32, 0:w], ps1[:, 0:w], ACTF.Silu)
                ps2 = psn.tile([32, 512], F32, tag="psn")
                nc.tensor.matmul(ps2[:, 0:w], lhsT=W(f"N2{l}"), rhs=n1[:, 0:w])
                nc.vector.tensor_tensor(hT[0:32, sl], hT[0:32, sl], ps2[:, 0:w],
                                        op=ALU.add)

        # ================= final =================
        for o, w in CHUNKS:
            sl = slice(o, o + w)
            ps = psn.tile([32, 512], F32, tag="psn")
            nc.tensor.matmul(ps[:, 0:w], lhsT=W("EOUT"), rhs=hT[:, sl])
            nc.scalar.copy(aggT[0:32, sl], ps[:, 0:w])
        psg = psphi.tile([32, G], F32, tag="phps")
        for b in range(BBLK):
            gohb = tp.tile([128, G], F32, tag="gohb")
            nc.sync.dma_start(gohb[:], ins["goh"][:, b * G:(b + 1) * G])
            t = psn.tile([128, 32], F32, tag="psn")
            nc.tensor.transpose(t[:, 0:32], aggT[0:32, b * 128:(b + 1) * 128],
                                ident[0:32, 0:32])
            onm = tp.tile([128, 32], F32, tag="n1")
            nc.scalar.copy(onm[:], t[:, 0:32])
            nc.tensor.matmul(psg[:], lhsT=onm[:], rhs=gohb[:],
                             start=(b == 0), stop=(b == BBLK - 1))
        gsb = wp.tile([32, G], F32, tag="gsb")
        nc.scalar.copy(gsb[:], psg[:])
        nc.sync.dma_start(gsin[:], gsb[:])
        nc.gpsimd.collective_compute(
            "AllReduce", ALU.add, replica_groups=[list(range(NC))],
            ins=[gsin[:]], outs=[gsout[:]],
        )
        hg = wp.tile([33, G], F32, tag="hg")
        nc.vector.memset(hg[32:33, :], 1.0)
        nc.sync.dma_start(hg[0:32, :], gsout[:])
        pm = psn.tile([32, G], F32, tag="psn")
        nc.tensor.matmul(pm[:], lhsT=W("ZMU"), rhs=hg[:])
        msb = wp.tile([32, G], F32, tag="msb")
        nc.scalar.copy(msb[:], pm[:])
        nc.sync.dma_start(mu_o[:], msb[:])
        ps2 = psn.tile([32, G], F32, tag="psn")
        nc.tensor.matmul(ps2[:], lhsT=W("ZSIG"), rhs=hg[:])
        # softplus(x) = max(x,0) + ln(1 + exp(-|x|))
        zsb = wp.tile([32, G], F32, tag="zsb")
        nc.scalar.copy(zsb[:], ps2[:])
        axp = wp.tile([32, G], F32, tag="axp")
        nc.scalar.activation(axp[:], zsb[:], ACTF.Abs)
        nc.scalar.activation(axp[:], axp[:], ACTF.Exp, scale=-1.0)
        nc.vector.tensor_scalar_add(axp[:], axp[:], 1.0)
        nc.scalar.activation(axp[:], axp[:], ACTF.Ln)
        ssb = wp.tile([32, G], F32, tag="ssb")
        nc.vector.tensor_scalar(ssb[:], zsb[:], 0.0, None, op0=ALU.max)
        nc.vector.tensor_tensor(ssb[:], ssb[:], axp[:], op=ALU.add)
        nc.sync.dma_start(sig_o[:], ssb[:])


# ----------------------------------------------------------------------
# kernel() entry point: full inputs -> full outputs, compiled + run on the
# 8 NeuronCores via run_bass_kernel_spmd (SPMD, one NEFF).
# ----------------------------------------------------------------------
import os

_CACHE = {}


def kernel(**inputs):
    import numpy as np
    from concourse import bacc
    from concourse.tile import TileContext
    from concourse.bass_utils import run_bass_kernel_spmd

    N = int(inputs["x"].shape[0])
    E = int(inputs["edge_index"].shape[1])
    G = 64
    lval = float(np.asarray(inputs["l"]).reshape(-1)[0])

    cfg = Cfg(N=N, E=E, G=G, L=lval)
    per_core = preprocess(inputs, cfg)
    specs = input_specs(cfg)

    key = (N, E, G, cfg.S1, cfg.S2, cfg.WC)
    if key in _CACHE:
        nc = _CACHE[key]
    else:
        nc = bacc.Bacc("TRN2", target_bir_lowering=False, debug=False,
                       num_devices=NC)
        ins = {}
        for k, (shape, dt) in specs.items():
            mdt = {np.int16: mybir.dt.int16, np.float32: mybir.dt.float32}[dt]
            ins[k] = nc.dram_tensor(k, list(shape), mdt, kind="ExternalInput").ap()
        outs = {
            "mu": nc.dram_tensor("mu", [32, G], F32, kind="ExternalOutput").ap(),
            "sig": nc.dram_tensor("sig", [32, G], F32, kind="ExternalOutput").ap(),
        }
        with TileContext(nc) as tc:
            build(tc, outs, ins, cfg)
        nc.compile()
        _CACHE[key] = nc

    in_maps = []
    for c in range(NC):
        m = {}
        for k, (shape, dt) in specs.items():
            a = np.ascontiguousarray(per_core[c][k]).astype(dt)
            assert a.shape == tuple(shape), (k, a.shape, shape)
            m[k] = a
        in_maps.append(m)

    res = run_bass_kernel_spmd(nc, in_maps, core_ids=list(range(NC)))
    r0 = res.results[0]
    mu = np.ascontiguousarray(r0["mu"].T)
    sigma = np.ascontiguousarray(r0["sig"].T)

    iters = int(os.environ.get("EGNN_BENCH", "0"))
    if iters > 0:
        _bench_pjrt(nc, in_maps, NC, iters)
    return mu, sigma


def _bench_pjrt(nc, in_maps, n_cores, iters):
    """Measure per-execution device time by pipelining batches of
    executions through PJRT and fitting the slope (amortizes the large
    fixed dispatch overhead of the remote device path)."""
    import time
    import numpy as np
    import jax
    from jax.sharding import Mesh, PartitionSpec, NamedSharding
    from jax.experimental.shard_map import shard_map
    import concourse.mybir as mybir
    from concourse import bass2jax
    from concourse.bass2jax import _bass_exec_p, partition_id_tensor

    bass2jax.install_neuronx_cc_hook()
    partition_name = (nc.partition_id_tensor.name
                      if nc.partition_id_tensor else None)
    in_names, out_names, out_avals, zero_outs = [], [], [], []
    for alloc in nc.m.functions[0].allocations:
        if not isinstance(alloc, mybir.MemoryLocationSet):
            continue
        name = alloc.memorylocations[0].name
        if alloc.kind == "ExternalInput":
            if name != partition_name:
                in_names.append(name)
        elif alloc.kind == "ExternalOutput":
            shape = tuple(alloc.tensor_shape)
            dtype = mybir.dt.np(alloc.dtype)
            out_names.append(name)
            out_avals.append(jax.core.ShapedArray(shape, dtype))
            zero_outs.append(np.zeros(shape, dtype))
    n_params = len(in_names)
    all_in_names = list(in_names) + list(out_names)
    if partition_name is not None:
        all_in_names.append(partition_name)

    def _body(*args):
        operands = list(args)
        if partition_name is not None:
            operands.append(partition_id_tensor())
        outs = _bass_exec_p.bind(
            *operands, out_avals=tuple(out_avals),
            in_names=tuple(all_in_names), out_names=tuple(out_names),
            lowering_input_output_aliases=(),
            sim_require_finite=True, sim_require_nnan=True, nc=nc)
        return tuple(outs)

    devices = jax.devices()[:n_cores]
    mesh = Mesh(np.asarray(devices), ("core",))
    spec = PartitionSpec("core")
    fn = jax.jit(shard_map(_body, mesh=mesh,
                           in_specs=(spec,) * (n_params + len(out_names)),
                           out_specs=(spec,) * len(out_names),
                           check_rep=False), keep_unused=True)
    sh = NamedSharding(mesh, spec)
    concat_in = [
        jax.device_put(
            np.concatenate([np.asarray(in_maps[c][nm]) for c in range(n_cores)],
                           axis=0), sh)
        for nm in in_names
    ]
    concat_zero = [
        jax.device_put(np.zeros((n_cores * z.shape[0], *z.shape[1:]), z.dtype), sh)
        for z in zero_outs
    ]
    outs = fn(*concat_in, *concat_zero)
    jax.block_until_ready(outs)  # warm compile + first exec

    def run_batch(k):
        t0 = time.perf_counter()
        outs = [fn(*concat_in, *concat_zero) for _ in range(k)]
        jax.block_until_ready(outs)
        return time.perf_counter() - t0

    k_lo, k_hi = 4, 4 + 24 * max(1, iters)
    t_lo = min(run_batch(k_lo) for _ in range(3))
    t_hi = min(run_batch(k_hi) for _ in range(3))
    per_exec = (t_hi - t_lo) / (k_hi - k_lo)
    print(f"batch timing: T({k_lo})={t_lo*1e3:.1f}ms T({k_hi})={t_hi*1e3:.1f}ms")
    print(f"HW exec time: {int(per_exec * 1e9)} ns")
